# revision 7
# baseline (speedup 1.0000x reference)
"""Trainium2 Bass kernel for nn_Encoder_Postnet (ragged_sequence).

Computation (reference):
    idx   = sequential aligner scan over (align_phone, text_phone)   [B,T]
    out   = enc[idx] + pitch @ w_pitch + b_pitch + emb_beats[beats]
            + (enc[idx] + pe) @ w_pos + b_pos

Key algebraic restructure: the duration-expansion gather commutes with the
E x E linear, so
    out[t] = encG[idx_t] + (pe @ w_pos)[t] + pitch[t]*w_pitch + beats[t]*demb + bias
with encG = enc @ (I + w_pos) computed once per batch row ([P,E] not [T,E]),
collapsing the big [B*T,E]@[E,E] matmul 8x and making the kernel memory-bound.

Sharding: pure data parallel, 2 batch rows per core across 8 cores.

Device pipeline per core:
  phase A: encG = enc_row @ (I+w_pos) via PE (fp32), staged to DRAM scratch.
  phase B: duration-expand encG rows to tokens, add (pe@w_pos) tile (DVE),
           add pitch/beats/bias terms via a tiny K=3 PE matmul into PSUM,
           DVE add, DMA out.

The aligner scan is index metadata ([B,T] int32); it is computed on host with
a run-compressed O(B*P) algorithm (exactly equivalent to the reference scan).
Two device programs exist:
  - fast path: when idx == arange(T)//8 for every row (the uniform duration-8
    expansion this model produces), the expansion is a static step-0
    replication access pattern on a plain HWDGE DMA - no gather instruction.
  - general path: arbitrary idx, via per-128-token indirect DMA row gathers
    (production-shaped offset [128,1] DynamicAP descriptors).
"""

import sys

for _p in ("/opt/trn_rl_repo",):
    if _p not in sys.path:
        sys.path.insert(0, _p)

import numpy as np

B, P, T, E = 16, 1024, 8192, 256
NCORES = 8
RPC = B // NCORES          # batch rows per core
CHUNK = 2048               # tokens per expansion chunk (fast path)
NCHUNK = T // CHUNK        # 4
NB = CHUNK // 128          # 16 token-blocks per chunk
NGRP = T // 128            # 64 groups of 128 tokens per row
DUR = T // P               # uniform duration of the fast path (8)

FORCE_GENERAL = False      # test hook: force the arbitrary-idx path
_CACHE = {}


# --------------------------------------------------------------------------
# Host: aligner index computation (exact replica of the reference recurrence)
# --------------------------------------------------------------------------
def compute_idx(align, text):
    """idx[b,0]=0; idx[b,j] = idx[b,j-1] if align[b,j]==text[b,idx[b,j-1]]
    else min(idx[b,j-1]+1, P-1).   Vectorized over batch via segment starts:
    the pointer advances i->i+1 at s_{i+1} = first j >= s_i+1 with
    align[j] != text[i]; within a run of align values equal to text[i] the
    first mismatch is the run end."""
    align = np.asarray(align)
    text = np.asarray(text)
    Bn, Tn = align.shape
    Pn = text.shape[1]
    diff = align[:, 1:] != align[:, :-1]                       # [B, T-1]
    c = np.full((Bn, Tn), Tn, np.int64)
    c[:, :-1] = np.where(diff, np.arange(1, Tn)[None, :], Tn)
    re = np.flip(np.minimum.accumulate(np.flip(c, axis=1), axis=1), axis=1)

    s = np.full((Bn, Pn), Tn, np.int64)
    s[:, 0] = 0
    cur = np.zeros(Bn, np.int64)
    arB = np.arange(Bn)
    for i in range(Pn - 1):
        j0 = cur + 1
        active = j0 < Tn
        j0c = np.minimum(j0, Tn - 1)
        eq = (align[arB, j0c] == text[:, i]) & active
        nxt = np.where(active, np.where(eq, re[arB, j0c], j0), Tn)
        s[:, i + 1] = nxt
        cur = nxt
    idx = np.empty((Bn, Tn), np.int32)
    pos = np.arange(Tn)
    for b in range(Bn):
        idx[b] = (np.searchsorted(s[b], pos, side="right") - 1).astype(np.int32)
    return idx


def _positional_encoding_f64(t, e):
    pos = np.arange(t, dtype=np.float64)[:, None]
    div = np.exp(np.arange(0, e, 2, dtype=np.float64) * (-np.log(10000.0) / e))
    ang = pos * div[None, :]
    return np.stack([np.sin(ang), np.cos(ang)], axis=-1).reshape(t, e)


def _block_perm(v):
    """Fast-path token permutation: token t -> (c=t//CHUNK, p=(t%CHUNK)//NB,
    n=t%NB), stored as [group gi=c*NB+n][128 p]."""
    return v.reshape(NCHUNK, 128, NB).swapaxes(1, 2).reshape(-1)


# --------------------------------------------------------------------------
# Device programs
# --------------------------------------------------------------------------
def _common_prelude(fast):
    import concourse.tile as tile
    from concourse import bacc, mybir
    from concourse._compat import get_trn_type

    f32 = mybir.dt.float32
    nc = bacc.Bacc(get_trn_type() or "TRN2", target_bir_lowering=False, debug=False)
    enc_t = nc.declare_dram_parameter("enc_t", [RPC, E, P], f32, isOutput=False)
    g_mat = nc.declare_dram_parameter("g_mat", [E, E], f32, isOutput=False)
    pe_w = nc.declare_dram_parameter("pe_w", [128, NGRP, E], f32, isOutput=False)
    p3 = nc.declare_dram_parameter("p3", [RPC, 3, T], f32, isOutput=False)
    w3 = nc.declare_dram_parameter("w3", [3, E], f32, isOutput=False)
    if not fast:
        idxo = nc.declare_dram_parameter(
            "idxo", [RPC, 128, NGRP], mybir.dt.int32, isOutput=False
        )
    else:
        idxo = None
    out = nc.declare_dram_parameter("out", [RPC, T, E], f32, isOutput=True)
    encg = nc.dram_tensor("encg", [RPC, P, E], f32)
    return nc, tile, mybir, (enc_t, g_mat, pe_w, p3, w3, idxo, out, encg)


def _emit_phase_a(nc, pools, tensors, f32):
    (enc_t, g_mat, pe_w, p3, w3, idxo, out, encg) = tensors
    const, encT_pool, psum_pool, eg_pool = pools
    g0 = const.tile([128, E], f32, tag="g0")
    g1 = const.tile([128, E], f32, tag="g1")
    nc.sync.dma_start(g0[:], g_mat[0:128, :])
    nc.sync.dma_start(g1[:], g_mat[128:256, :])
    w3_sb = const.tile([3, E], f32, tag="w3")
    nc.sync.dma_start(w3_sb[:], w3[:, :])
    p3_sb = []
    for r in range(RPC):
        p3t = const.tile([3, T], f32, tag=f"p3_{r}")
        nc.sync.dma_start(p3t[:], p3[r])
        p3_sb.append(p3t)

    for r in range(RPC):
        et0 = encT_pool.tile([128, P], f32, tag="et0")
        et1 = encT_pool.tile([128, P], f32, tag="et1")
        nc.sync.dma_start(et0[:], enc_t[r, 0:128, :])
        nc.sync.dma_start(et1[:], enc_t[r, 128:256, :])
        ps = psum_pool.tile([128, 8 * E], f32, tag="ps")
        for m in range(8):
            nc.tensor.matmul(
                ps[:, m * E:(m + 1) * E],
                lhsT=et0[:, m * 128:(m + 1) * 128],
                rhs=g0[:],
                start=True, stop=False,
            )
            nc.tensor.matmul(
                ps[:, m * E:(m + 1) * E],
                lhsT=et1[:, m * 128:(m + 1) * 128],
                rhs=g1[:],
                start=False, stop=True,
            )
        eg = eg_pool.tile([128, 8 * E], f32, tag="eg")
        nc.vector.tensor_copy(eg[:], ps[:])
        nc.sync.dma_start(
            encg[r].rearrange("(m p) e -> p m e", p=128),
            eg[:].rearrange("q (m e) -> q m e", e=E),
        )
    return w3_sb, p3_sb


def build_nc_fast():
    """Static duration-8 expansion path."""
    import concourse.bass as bass
    from contextlib import ExitStack

    nc, tile, mybir, tensors = _common_prelude(fast=True)
    (enc_t, g_mat, pe_w, p3, w3, idxo, out, encg) = tensors
    f32 = mybir.dt.float32

    with tile.TileContext(nc) as tc, ExitStack() as ctx:
        const = ctx.enter_context(tc.tile_pool(name="const", bufs=1))
        encT_pool = ctx.enter_context(tc.tile_pool(name="encT", bufs=2))
        psum_pool = ctx.enter_context(tc.tile_pool(name="psum", bufs=2, space="PSUM"))
        eg_pool = ctx.enter_context(tc.tile_pool(name="eg", bufs=2))
        pe_pool = ctx.enter_context(tc.tile_pool(name="pe", bufs=2))
        gath_pool = ctx.enter_context(tc.tile_pool(name="gath", bufs=3))

        w3_sb, p3_sb = _emit_phase_a(
            nc, (const, encT_pool, psum_pool, eg_pool), tensors, f32
        )

        encg_h = encg[:].tensor
        for c in range(NCHUNK):
            pe_t = pe_pool.tile([128, NB, E], f32, tag="pe")
            nc.sync.dma_start(pe_t[:], pe_w[:, c * NB:(c + 1) * NB, :])
            for r in range(RPC):
                gt = gath_pool.tile([128, NB, E], f32, tag="gt")
                # expansion: gt[p, n1*8+n0, e] = encg[r, c*256 + 2p + n1, e]
                src_exp = bass.AP(
                    encg_h,
                    (r * P + c * (CHUNK // DUR)) * E,
                    [[2 * E, 128], [E, 2], [0, DUR], [1, E]],
                )
                nc.sync.dma_start(
                    gt[:].rearrange("q (a b) e -> q a b e", a=2), src_exp
                )
                # gt += pe@w_pos tile (one DVE op over the whole chunk)
                nc.vector.tensor_add(gt[:], gt[:], pe_t[:])
                for h in range(2):
                    ps = psum_pool.tile([128, 8 * E], f32, tag="ps")
                    for g in range(8):
                        gi = c * NB + h * 8 + g
                        nc.tensor.matmul(
                            ps[:, g * E:(g + 1) * E],
                            lhsT=p3_sb[r][:, gi * 128:(gi + 1) * 128],
                            rhs=w3_sb[:],
                            start=True, stop=True,
                        )
                    nc.vector.tensor_add(
                        gt[:, h * 8:(h + 1) * 8, :],
                        gt[:, h * 8:(h + 1) * 8, :],
                        ps[:].rearrange("q (n e) -> q n e", e=E),
                    )
                nc.sync.dma_start(
                    out[r, c * CHUNK:(c + 1) * CHUNK, :].rearrange(
                        "(p n) e -> p n e", n=NB
                    ),
                    gt[:],
                )
    nc.compile()
    return nc


def build_nc_general():
    """Arbitrary-idx path: per-128-token indirect row gathers."""
    import concourse.bass as bass
    from contextlib import ExitStack

    nc, tile, mybir, tensors = _common_prelude(fast=False)
    (enc_t, g_mat, pe_w, p3, w3, idxo, out, encg) = tensors
    f32 = mybir.dt.float32
    i32 = mybir.dt.int32

    with tile.TileContext(nc) as tc, ExitStack() as ctx:
        const = ctx.enter_context(tc.tile_pool(name="const", bufs=1))
        encT_pool = ctx.enter_context(tc.tile_pool(name="encT", bufs=2))
        psum_pool = ctx.enter_context(tc.tile_pool(name="psum", bufs=2, space="PSUM"))
        eg_pool = ctx.enter_context(tc.tile_pool(name="eg", bufs=2))
        pe_pool = ctx.enter_context(tc.tile_pool(name="pe", bufs=2))
        gath_pool = ctx.enter_context(tc.tile_pool(name="gath", bufs=3))

        w3_sb, p3_sb = _emit_phase_a(
            nc, (const, encT_pool, psum_pool, eg_pool), tensors, f32
        )
        ixo_sb = []
        for r in range(RPC):
            ixt = const.tile([128, NGRP], i32, tag=f"ixo_{r}")
            nc.sync.dma_start(ixt[:], idxo[r])
            ixo_sb.append(ixt)

        encg_flat = encg[:].rearrange("r p e -> (r p) e")
        NSUP = 8                # token groups per super-chunk
        for s in range(T // (NSUP * 128)):
            pe_t = pe_pool.tile([128, NSUP, E], f32, tag="pe")
            nc.sync.dma_start(
                pe_t[:], pe_w[:, s * NSUP:(s + 1) * NSUP, :]
            )
            for r in range(RPC):
                gt = gath_pool.tile([128, NSUP, E], f32, tag="gt")
                for g in range(NSUP):
                    gi = s * NSUP + g
                    nc.gpsimd.indirect_dma_start(
                        out=gt[:, g, :],
                        out_offset=None,
                        in_=encg_flat,
                        in_offset=bass.IndirectOffsetOnAxis(
                            ap=ixo_sb[r][:, gi:gi + 1], axis=0
                        ),
                    )
                nc.vector.tensor_add(gt[:], gt[:], pe_t[:])
                ps = psum_pool.tile([128, 8 * E], f32, tag="ps")
                for g in range(NSUP):
                    gi = s * NSUP + g
                    nc.tensor.matmul(
                        ps[:, g * E:(g + 1) * E],
                        lhsT=p3_sb[r][:, gi * 128:(gi + 1) * 128],
                        rhs=w3_sb[:],
                        start=True, stop=True,
                    )
                nc.vector.tensor_add(
                    gt[:], gt[:], ps[:].rearrange("q (n e) -> q n e", e=E)
                )
                nc.sync.dma_start(
                    out[r, s * NSUP * 128:(s + 1) * NSUP * 128, :].rearrange(
                        "(n p) e -> p n e", p=128
                    ),
                    gt[:],
                )
    nc.compile()
    return nc


def get_nc(fast):
    key = "nc_fast" if fast else "nc_gen"
    if key not in _CACHE:
        _CACHE[key] = build_nc_fast() if fast else build_nc_general()
    return _CACHE[key]


# --------------------------------------------------------------------------
# Host wrapper
# --------------------------------------------------------------------------
def make_in_maps(encoder_out, align_phone, text_phone, pitch, beats,
                 w_pitch, b_pitch, emb_beats, w_pos, b_pos):
    encoder_out = np.asarray(encoder_out, np.float32)
    pitch = np.asarray(pitch, np.float32)
    beats = np.asarray(beats)
    w_pitch = np.asarray(w_pitch, np.float32)
    w_pos = np.asarray(w_pos, np.float32)

    idx = compute_idx(np.asarray(align_phone), np.asarray(text_phone))
    fast = bool(np.all(idx == (np.arange(T, dtype=np.int32) // DUR)[None, :]))
    if FORCE_GENERAL:
        fast = False

    g_mat = (np.eye(E, dtype=np.float64) + w_pos.astype(np.float64)).astype(np.float32)
    pe = _positional_encoding_f64(T, E)
    pe_proj = (pe @ w_pos.astype(np.float64)).astype(np.float32)     # [T, E]
    if fast:
        pe_wl = pe_proj.reshape(NCHUNK, 128, NB, E).swapaxes(0, 1).reshape(128, NGRP, E)
    else:
        pe_wl = pe_proj.reshape(NGRP, 128, E).swapaxes(0, 1)         # [128, NGRP, E]
    pe_wl = np.ascontiguousarray(pe_wl)
    bias = (np.asarray(emb_beats[0], np.float64)
            + np.asarray(b_pitch, np.float64)
            + np.asarray(b_pos, np.float64))
    demb = np.asarray(emb_beats[1], np.float64) - np.asarray(emb_beats[0], np.float64)
    w3 = np.stack([w_pitch[0].astype(np.float64), demb, bias]).astype(np.float32)

    in_maps = []
    for core in range(NCORES):
        rows = range(core * RPC, (core + 1) * RPC)
        enc_t = np.ascontiguousarray(
            encoder_out[core * RPC:(core + 1) * RPC].transpose(0, 2, 1)
        )
        p3 = np.empty((RPC, 3, T), np.float32)
        idxo = np.empty((RPC, 128, NGRP), np.int32)
        for j, b in enumerate(rows):
            if fast:
                p3[j, 0] = _block_perm(pitch[b, :, 0])
                p3[j, 1] = _block_perm(beats[b, :, 0].astype(np.float32))
            else:
                p3[j, 0] = pitch[b, :, 0]
                p3[j, 1] = beats[b, :, 0].astype(np.float32)
                idxo[j] = idx[b].reshape(NGRP, 128).T + j * P
            p3[j, 2] = 1.0
        m = {
            "enc_t": enc_t,
            "g_mat": g_mat,
            "pe_w": pe_wl,
            "p3": p3,
            "w3": w3,
        }
        if not fast:
            m["idxo"] = idxo
        in_maps.append(m)
    return fast, in_maps


def kernel(**inputs):
    from concourse.bass_utils import run_bass_kernel_spmd

    fast, in_maps = make_in_maps(**inputs)
    nc = get_nc(fast)
    res = run_bass_kernel_spmd(nc, in_maps, core_ids=list(range(NCORES)))
    out = np.concatenate([res.results[i]["out"] for i in range(NCORES)], axis=0)
    return np.ascontiguousarray(out.astype(np.float32))
